# revision 1
# baseline (speedup 1.0000x reference)
"""Trainium2 Bass kernel for a CMAE loss (masked reconstruction + contrastive).

Computes, for full inputs:
  reconstruct_loss = sum(mask * mean_P((pred - norm(target))^2)) / sum(mask)
      with norm(t) = (t - mean(t)) / sqrt(var_unbiased(t) + 1e-6)  per (b, l) row
  contrastive_loss = (sum_i logsumexp_j(S_ij/T) - trace(S)/T) / N
      with S = cos-sim matrix of row-normalized student/teacher [N, D]
  total = reconstruct_loss + contrastive_loss

Sharding: data-parallel over B across 8 NeuronCores (16 batches per core,
3136 rows of 768 pixels each); student/teacher (tiny) replicated, the
contrastive part computed identically on every core.  Each core emits
[num_partial, mask_partial, lse_sum, pos_sum]; the host sums partials and
forms the three scalars.

Per-core math (rows-on-partitions layout, [128, 768] tiles):
  per row we need Srt=sum(t), q=sum((t-m)^2), Sp=sum(p), Sp2=sum(p^2),
  Spt=sum(p*t); then
  768*loss = Sp2 - 2*inv*(Spt - m*Sp) + q*inv2,
  inv2 = 767/(q + 767e-6), inv = sqrt(inv2).
  Engines: DVE does bn_stats (mean/var of t in one pass) + fused
  fused scalar_tensor_tensor for Spt; ACT does Square/Copy with accum_out for
  Sp2/Sp.  Everything streams; DMA is the roofline (~19.3 MB/core of t
  plus p at ~360 GB/s).
"""

import numpy as np

B, L, P = 128, 196, 768
N, D = 128, 256
NCORES = 8
BSH = B // NCORES            # 16 batches per core
ROWS = BSH * L               # 3136 rows per core
NT = (ROWS + 127) // 128     # 25 tiles (24 full + one of 64 rows)
TEMP = 0.1
CP = float(P - 1)            # 767, unbiased-variance divisor
EPS_VAR = 1e-6

_CACHE = {}
ABLATE = set()  # {'dve','act'}: skip those recon-loop pieces (timing experiments)
RPC = 2           # rows per partition per chunk DMA (divisor of 24)
DMA_P = "sync"    # engine issuing pred loads: sync | scalar | gpsimd


def _build_program(repeat=1):
    import concourse.bacc as bacc
    import concourse.mybir as mybir
    import concourse.tile as tile
    from concourse.masks import make_identity

    class _Bacc(bacc.Bacc):
        """Bacc whose ACT-table chooser is restricted so every activation
        this kernel uses (Ln/Exp/Square/Copy/Identity) resolves to the one
        set that contains them all -- avoids ~6 ping-ponging table loads
        (~2.7us each) between natural_log / exp_and_others."""

        def insert_act_table_loads(self):
            from concourse.hw_specs import get_activation_tables
            import bass_rust as _br

            has_activation = any(
                isinstance(i, mybir.InstActivation)
                for b in self.main_func.blocks
                for i in b.instructions
            )
            if not has_activation:
                return
            mine = {
                mybir.ActivationFunctionType.Ln,
                mybir.ActivationFunctionType.Exp,
                mybir.ActivationFunctionType.Square,
                mybir.ActivationFunctionType.Copy,
                mybir.ActivationFunctionType.Identity,
            }
            keep = "natural_log_exp_and_others"
            tables = [
                (nm, (fs if nm == keep else (fs - mine)))
                for nm, fs in get_activation_tables(self.m.arch).items()
            ]
            _br.insert_act_table_loads(self, tables)

    f32 = mybir.dt.float32
    Alu = mybir.AluOpType
    Act = mybir.ActivationFunctionType
    X = mybir.AxisListType.X

    nc = _Bacc(
        "TRN2",
        target_bir_lowering=False,
        debug=False,
        enable_asserts=False,
    )
    tgt = nc.dram_tensor("target", [ROWS, P], f32, kind="ExternalInput").ap()
    prd = nc.dram_tensor("pred", [ROWS, P], f32, kind="ExternalInput").ap()
    msk = nc.dram_tensor("mask", [ROWS], f32, kind="ExternalInput").ap()
    stu = nc.dram_tensor("student", [N, D], f32, kind="ExternalInput").ap()
    tea = nc.dram_tensor("teacher", [N, D], f32, kind="ExternalInput").ap()
    out = nc.dram_tensor("out", [1, 4], f32, kind="ExternalOutput").ap()

    from contextlib import ExitStack

    with tile.TileContext(nc) as tc:
        with ExitStack() as ctx:
            consts = ctx.enter_context(tc.tile_pool(name="consts", bufs=1))
            accs = ctx.enter_context(tc.tile_pool(name="accs", bufs=1))
            io_t = ctx.enter_context(tc.tile_pool(name="io_t", bufs=4))
            io_p = ctx.enter_context(tc.tile_pool(name="io_p", bufs=4))
            scr_v = ctx.enter_context(tc.tile_pool(name="scr_v", bufs=2))
            scr_a = ctx.enter_context(tc.tile_pool(name="scr_a", bufs=2))
            small = ctx.enter_context(tc.tile_pool(name="small", bufs=2))
            epi = ctx.enter_context(tc.tile_pool(name="epi", bufs=1))
            psum = ctx.enter_context(tc.tile_pool(name="psum", bufs=2, space="PSUM"))
            # ---- constants / accumulators ----
            ident = consts.tile([128, 128], f32)
            make_identity(nc, ident)
            ones = consts.tile([128, 1], f32)
            nc.gpsimd.memset(ones, 1.0)
            zb = consts.tile([128, 1], f32)
            nc.gpsimd.memset(zb, 0.0)
            lnT = consts.tile([128, 1], f32)
            nc.gpsimd.memset(lnT, float(np.log(1.0 / TEMP)))

            # F columns: 0=masked-loss partial, 1=mask partial, 2=lse, 3=diag
            # (the body below is optionally repeated for benchmarking)
            rep_bodies = range(repeat)
            for _rep in rep_bodies:
                _run_body(
                    nc, tc, consts, accs, io_t, io_p, scr_v, scr_a, small, epi,
                    psum, tgt, prd, msk, stu, tea, out, ident, ones, zb, lnT,
                    mybir,
                )
    nc.compile()
    return nc


def _run_body(nc, tc, consts, accs, io_t, io_p, scr_v, scr_a, small, epi, psum,
              tgt, prd, msk, stu, tea, out, ident, ones, zb, lnT, mybir):
    import numpy as np

    f32 = mybir.dt.float32
    Alu = mybir.AluOpType
    Act = mybir.ActivationFunctionType
    X = mybir.AxisListType.X

    if True:
        if True:
            # F columns: 0=masked-loss partial, 1=mask partial, 2=lse, 3=diag
            F = accs.tile([128, 4], f32)
            nc.gpsimd.memset(F, 0.0)
            mv = accs.tile([128, NT, 2], f32)      # per-tile (mean, var) of t
            nc.gpsimd.memset(mv, 0.0)
            s_pt = accs.tile([128, NT], f32)
            nc.gpsimd.memset(s_pt, 0.0)
            s_p = accs.tile([128, NT], f32)
            nc.gpsimd.memset(s_p, 0.0)
            s_p2 = accs.tile([128, NT], f32)
            nc.gpsimd.memset(s_p2, 0.0)
            mask_sb = accs.tile([128, NT], f32)
            nc.gpsimd.memset(mask_sb, 0.0)

            # ---- reconstruction: block-row layout ----
            # Partition p holds rows RPB*p + j (j in [0, RPB)): each chunk
            # DMA moves RPC rows per partition as ONE contiguous RPC*3072B
            # descriptor per partition line (vs 3072B in row-per-partition
            # layout), which is what the DMA engines need to reach full
            # HBM bandwidth.  The last REM rows go one-per-partition into
            # stat column RPB.  A [128, 768] j-slice covers 128 rows, so
            # per-slice compute is identical to the old per-tile compute.
            half = P // 2
            RPB = ROWS // 128                   # 24 rows per partition
            REM = ROWS - 128 * RPB              # 64 remainder rows
            tgt_blk = tgt[0 : 128 * RPB].rearrange("(p j) d -> p j d", j=RPB)
            prd_blk = prd[0 : 128 * RPB].rearrange("(p j) d -> p j d", j=RPB)
            p_dma = getattr(nc, DMA_P)
            if REM:
                h = REM
                t_r = io_t.tile([128, P], f32, tag="tr")
                nc.sync.dma_start(out=t_r[:h], in_=tgt[128 * RPB : ROWS, :])
                p_r = io_p.tile([128, P], f32, tag="pr")
                p_dma.dma_start(out=p_r[:h], in_=prd[128 * RPB : ROWS, :])
                if "dve" not in ABLATE:
                    st = scr_v.tile([128, 2, 6], f32, tag="bn")
                    nc.vector.bn_stats(st[:h, 0, :], t_r[:h, 0:half])
                    nc.vector.bn_stats(st[:h, 1, :], t_r[:h, half:P])
                    nc.vector.bn_aggr(mv[:h, RPB, :], st[:h])
                    sv = scr_v.tile([128, P], f32, tag="sv")
                    nc.vector.scalar_tensor_tensor(
                        out=sv[:h], in0=t_r[:h], scalar=1.0, in1=p_r[:h],
                        op0=Alu.mult, op1=Alu.mult,
                        accum_out=s_pt[:h, RPB : RPB + 1],
                    )
                if "act" not in ABLATE:
                    sa = scr_a.tile([128, P], f32, tag="sa")
                    nc.scalar.activation(
                        sa[:h], p_r[:h], Act.Square, bias=zb[:h],
                        accum_out=s_p2[:h, RPB : RPB + 1],
                    )
                    sa2 = scr_a.tile([128, P], f32, tag="sa2")
                    nc.scalar.activation(
                        sa2[:h], p_r[:h], Act.Copy,
                        accum_out=s_p[:h, RPB : RPB + 1],
                    )

            nchunk = RPB // RPC
            for c in range(nchunk):
                j0 = c * RPC
                if c == 2:
                        # mask in block-row layout: mask_sb[p, j] = mask[RPB*p + j]
                        nfull = ROWS // 128                      # RPB = 24
                        rem = ROWS - nfull * 128                 # 64
                        nc.sync.dma_start(
                            out=mask_sb[:, 0:nfull],
                            in_=msk[0 : nfull * 128].rearrange("(p j) -> p j", j=nfull),
                        )
                        if rem:
                            nc.sync.dma_start(
                                out=mask_sb[0:rem, nfull : nfull + 1],
                                in_=msk[nfull * 128 : ROWS].rearrange("(p j) -> p j", j=1),
                            )

                        # ---- contrastive part (tiny, replicated on every core) ----
                        stu_sb = consts.tile([N, D], f32)
                        nc.sync.dma_start(out=stu_sb, in_=stu)
                        tea_sb = consts.tile([N, D], f32)
                        nc.sync.dma_start(out=tea_sb, in_=tea)

                        qs = small.tile([128, 1], f32)
                        qt = small.tile([128, 1], f32)
                        c_scr = small.tile([N, D], f32)
                        nc.vector.scalar_tensor_tensor(
                            out=c_scr, in0=stu_sb, scalar=1.0, in1=stu_sb,
                            op0=Alu.mult, op1=Alu.mult, accum_out=qs,
                        )
                        c_scr2 = small.tile([N, D], f32)
                        nc.vector.scalar_tensor_tensor(
                            out=c_scr2, in0=tea_sb, scalar=1.0, in1=tea_sb,
                            op0=Alu.mult, op1=Alu.mult, accum_out=qt,
                        )
                        # 1/||row|| = exp(-0.5*ln(q)); student side also folds in 1/T=10
                        lnqs = small.tile([128, 1], f32)
                        nc.scalar.activation(lnqs, qs, Act.Ln, bias=zb)
                        lnqt = small.tile([128, 1], f32)
                        nc.scalar.activation(lnqt, qt, Act.Ln, bias=zb)
                        a10 = small.tile([128, 1], f32)
                        nc.scalar.activation(a10, lnqs, Act.Exp, scale=-0.5, bias=lnT)
                        b1 = small.tile([128, 1], f32)
                        nc.scalar.activation(b1, lnqt, Act.Exp, scale=-0.5, bias=zb)

                        PN = consts.tile([N, D], f32)
                        nc.vector.tensor_scalar(
                            out=PN, in0=stu_sb, scalar1=a10, scalar2=None, op0=Alu.mult
                        )
                        TN = consts.tile([N, D], f32)
                        nc.vector.tensor_scalar(
                            out=TN, in0=tea_sb, scalar1=b1, scalar2=None, op0=Alu.mult
                        )
                        # diag of S: row-dots of the scaled matrices -> F[:, 3]
                        c_scr3 = small.tile([N, D], f32)
                        nc.vector.scalar_tensor_tensor(
                            out=c_scr3, in0=PN, scalar=1.0, in1=TN,
                            op0=Alu.mult, op1=Alu.mult, accum_out=F[:, 3:4],
                        )

                        # S = PN @ TN.T via PE: transpose both, then 2 accumulating matmuls
                        nchunks = D // 128
                        pnt = []
                        tnt = []
                        for c in range(nchunks):
                            for src, dstlist, nm in ((PN, pnt, "pn"), (TN, tnt, "tn")):
                                ps = psum.tile([128, 128], f32, tag="tr_ps")
                                nc.tensor.transpose(ps, src[:, c * 128 : (c + 1) * 128], ident)
                                sb = consts.tile([128, 128], f32, tag=f"{nm}t{c}")
                                nc.scalar.copy(sb, ps)
                                dstlist.append(sb)
                        S_ps = psum.tile([128, 128], f32, tag="S")
                        for c in range(nchunks):
                            nc.tensor.matmul(
                                S_ps, lhsT=pnt[c], rhs=tnt[c],
                                start=(c == 0), stop=(c == nchunks - 1),
                            )
                        # row-wise logsumexp -> F[:, 2]
                        rm_neg = small.tile([128, 1], f32)
                        nc.vector.tensor_reduce(rm_neg, S_ps, axis=X, op=Alu.max, negate=True)
                        E = small.tile([128, 128], f32)
                        sume = small.tile([128, 1], f32)
                        nc.scalar.activation(E, S_ps, Act.Exp, bias=rm_neg, accum_out=sume)
                        lnsum = small.tile([128, 1], f32)
                        nc.scalar.activation(lnsum, sume, Act.Ln, bias=zb)
                        nc.vector.tensor_sub(F[:, 2:3], lnsum, rm_neg)


                t_t = io_t.tile([128, RPC, P], f32, tag="t")
                nc.sync.dma_start(out=t_t, in_=tgt_blk[:, j0 : j0 + RPC, :])
                p_t = io_p.tile([128, RPC, P], f32, tag="p")
                p_dma.dma_start(out=p_t, in_=prd_blk[:, j0 : j0 + RPC, :])
                for jj in range(RPC):
                    j = j0 + jj
                    if "dve" not in ABLATE:
                        st = scr_v.tile([128, 2, 6], f32, tag="bn")
                        nc.vector.bn_stats(st[:, 0, :], t_t[:, jj, 0:half])
                        nc.vector.bn_stats(st[:, 1, :], t_t[:, jj, half:P])
                        nc.vector.bn_aggr(mv[:, j, :], st)
                        sv = scr_v.tile([128, P], f32, tag="sv")
                        nc.vector.scalar_tensor_tensor(
                            out=sv, in0=t_t[:, jj, :], scalar=1.0,
                            in1=p_t[:, jj, :], op0=Alu.mult, op1=Alu.mult,
                            accum_out=s_pt[:, j : j + 1],
                        )
                    if "act" not in ABLATE:
                        sa = scr_a.tile([128, P], f32, tag="sa")
                        nc.scalar.activation(
                            sa, p_t[:, jj, :], Act.Square, bias=zb,
                            accum_out=s_p2[:, j : j + 1],
                        )
                        sa2 = scr_a.tile([128, P], f32, tag="sa2")
                        nc.scalar.activation(
                            sa2, p_t[:, jj, :], Act.Copy,
                            accum_out=s_p[:, j : j + 1],
                        )

            # ---- per-row loss epilogue on the [128, NT] stat buffers ----
            m_ap = mv[:, :, 0]
            vp_ap = mv[:, :, 1]
            QE = epi.tile([128, NT], f32)   # q + 767*eps, q = P*var_pop
            nc.vector.tensor_scalar(
                out=QE, in0=vp_ap, scalar1=float(P), scalar2=CP * EPS_VAR,
                op0=Alu.mult, op1=Alu.add,
            )
            R = epi.tile([128, NT], f32)
            nc.vector.reciprocal(R, QE)
            # inv = sqrt(767*R) = exp(0.5*ln(767*R))
            LNR = epi.tile([128, NT], f32)
            nc.scalar.activation(LNR, R, Act.Ln, scale=CP, bias=zb)
            INV = epi.tile([128, NT], f32)
            nc.scalar.activation(INV, LNR, Act.Exp, scale=0.5, bias=zb)
            CRA = epi.tile([128, NT], f32)
            nc.vector.tensor_mul(CRA, m_ap, s_p)
            CRS = epi.tile([128, NT], f32)
            nc.vector.tensor_sub(CRS, s_pt, CRA)        # cross = Spt - m*Sp
            T1 = epi.tile([128, NT], f32)
            nc.vector.tensor_mul(T1, INV, CRS)
            T2 = epi.tile([128, NT], f32)
            nc.vector.scalar_tensor_tensor(
                out=T2, in0=T1, scalar=-2.0, in1=s_p2, op0=Alu.mult, op1=Alu.add
            )
            T3 = epi.tile([128, NT], f32)
            nc.vector.tensor_mul(T3, vp_ap, R)
            T4 = epi.tile([128, NT], f32)   # = 768 * per-row loss
            nc.vector.scalar_tensor_tensor(
                out=T4, in0=T3, scalar=float(P) * CP, in1=T2,
                op0=Alu.mult, op1=Alu.add,
            )
            LM = epi.tile([128, NT], f32)
            nc.vector.scalar_tensor_tensor(
                out=LM, in0=T4, scalar=1.0 / P, in1=mask_sb,
                op0=Alu.mult, op1=Alu.mult, accum_out=F[:, 0:1],
            )
            nc.vector.tensor_reduce(F[:, 1:2], mask_sb, axis=X, op=Alu.add)

            # ---- cross-partition reduction of F via ones-matmul ----
            F_ps = psum.tile([1, 4], f32, tag="fout")
            nc.tensor.matmul(F_ps, lhsT=ones, rhs=F, start=True, stop=True)
            outF = small.tile([1, 4], f32)
            nc.scalar.copy(outF, F_ps)
            nc.sync.dma_start(out=out, in_=outF)


def _get_program(repeat=1):
    key = ("nc", repeat, tuple(sorted(ABLATE)), RPC, DMA_P)
    if key not in _CACHE:
        _CACHE[key] = _build_program(repeat)
    return _CACHE[key]


def _shard_inputs(student_prob, teacher_prob, reconstruct_target, reconstruct_pred, mask):
    student = np.ascontiguousarray(student_prob, dtype=np.float32)
    teacher = np.ascontiguousarray(teacher_prob, dtype=np.float32)
    tgt = np.ascontiguousarray(reconstruct_target, dtype=np.float32)
    prd = np.ascontiguousarray(reconstruct_pred, dtype=np.float32)
    msk = np.ascontiguousarray(mask, dtype=np.float32)

    in_maps = []
    for c in range(NCORES):
        sl = slice(c * BSH, (c + 1) * BSH)
        in_maps.append(
            {
                "target": tgt[sl].reshape(ROWS, P),
                "pred": prd[sl].reshape(ROWS, P),
                "mask": msk[sl].reshape(ROWS),
                "student": student,
                "teacher": teacher,
            }
        )
    return in_maps


def _combine(results):
    outs = np.stack([r["out"][0] for r in results])  # [NCORES, 4]
    num = float(outs[:, 0].sum())
    den = float(outs[:, 1].sum())
    recon = num / den
    contr = (float(outs[0, 2]) - float(outs[0, 3])) / N
    total = recon + contr
    return (np.float32(recon), np.float32(contr), np.float32(total))


def run(in_maps, repeat=1, **kwargs):
    from concourse.bass_utils import run_bass_kernel_spmd

    nc = _get_program(repeat)
    return run_bass_kernel_spmd(nc, in_maps, core_ids=list(range(NCORES)), **kwargs)


def kernel(student_prob, teacher_prob, reconstruct_target, reconstruct_pred, mask):
    in_maps = _shard_inputs(
        student_prob, teacher_prob, reconstruct_target, reconstruct_pred, mask
    )
    res = run(in_maps)
    return _combine(res.results)

